# revision 18
# baseline (speedup 1.0000x reference)
"""Trainium2 Bass kernel for the attention-weighted pair-combine module.

Reference math (per row i of N):
    w1 = leaky_relu(x1 @ W1 + b1, 0.01) @ W2 + b2     [scalar]
    w2 = leaky_relu(x2 @ W1 + b1, 0.01) @ W2 + b2     [scalar]
    (b1, b2) = softmax over {w1, w2}
    Z = tanh([b1*x1, b2*x2])   -> [N, 1, 2C]
    beta = [b1; b2]            -> [N, 2, 1]

Device strategy (pure data-parallel over 8 NeuronCores, see spec sharding_hint):
  - softmax over 2 == sigmoid of the difference: beta1 = 0.5*tanh(0.5*(w1-w2)) + 0.5.
    b2 cancels entirely; only Tanh ACT tables are used -> no table switches.
  - leaky_relu(u)@W2 == 0.01*u@W2 + 0.99*relu(u)@W2. The linear term is x @ g
    with g = 0.01*W1@W2 folded as an extra COLUMN of the v-matmul weights
    (wpaug = [W1 | g]), so it costs nothing. The relu-evac uses a per-partition
    max-floor (0 for relu rows, -3e38 for the linear row) so one fused DVE
    tensor_scalar evacuates relu(v+b1) AND passes the linear row through.
  - scores for x2 use host-negated weights and PSUM-accumulate on top of x1's,
    so the score PSUM holds d = 0.5*(w1-w2) directly.
  - x is pre-cast to bf16 on host; the transposed copy (needed because the PE
    contracts over the partition dim) is loaded by DMA-transpose, which needs
    a 2-byte dtype. No PE transposes, no on-chip casts.
  - rows are tiled 2048 at a time; partition p owns 16 consecutive rows of the
    tile (block mapping) so every DMA descriptor is >= 128B contiguous.
"""

import numpy as np
import ml_dtypes
from contextlib import ExitStack

N_TOTAL = 500000
C = 128
H = 64
HA = H + 1        # v rows: 64 relu features + 1 linear passthrough row
NCORES = 8
P = 128           # partitions
T = 16            # rows per partition per tile (block mapping)
TILE_ROWS = P * T # 2048
SHARD = N_TOTAL // NCORES             # 62500
NTILES = -(-SHARD // TILE_ROWS)       # 31
ROWS_PAD = NTILES * TILE_ROWS         # 63488
SUBC = 4          # chunks (of 128 rows) per PE sub-pass; 4*128 = 512 moving cols


def _build(ntiles):
    import concourse.tile as tile
    from concourse import bacc, mybir

    f32 = mybir.dt.float32
    bf16 = mybir.dt.bfloat16
    AF = mybir.ActivationFunctionType
    ALU = mybir.AluOpType

    rows = ntiles * TILE_ROWS
    # Bacc (not plain Bass): its compile() legalizes multi-sem-wait
    # instructions, which this walrus build rejects outright.
    nc = bacc.Bacc(None, debug=False)
    x1d = nc.dram_tensor("x1b", [rows, C], bf16, kind="ExternalInput")
    x2d = nc.dram_tensor("x2b", [rows, C], bf16, kind="ExternalInput")
    wpd = nc.dram_tensor("wpaug", [C, HA], bf16, kind="ExternalInput")
    w2d = nc.dram_tensor("w2s", [HA, 2], bf16, kind="ExternalInput")
    bfd = nc.dram_tensor("bfl", [HA, 2], f32, kind="ExternalInput")
    zd = nc.dram_tensor("z", [rows, 2 * C], f32, kind="ExternalOutput")
    bd = nc.dram_tensor("beta", [rows, 2], f32, kind="ExternalOutput")

    with ExitStack() as ctx:
        tc = ctx.enter_context(tile.TileContext(nc))
        const = ctx.enter_context(tc.tile_pool(name="const", bufs=1))
        xin = ctx.enter_context(tc.tile_pool(name="xin", bufs=3))
        xtp = ctx.enter_context(tc.tile_pool(name="xt", bufs=3))
        relup = ctx.enter_context(tc.tile_pool(name="relu", bufs=2))
        small = ctx.enter_context(tc.tile_pool(name="small", bufs=2))
        zp = ctx.enter_context(tc.tile_pool(name="zp", bufs=3))
        psV = ctx.enter_context(tc.tile_pool(name="psV", bufs=3, space="PSUM"))
        psS = ctx.enter_context(tc.tile_pool(name="psS", bufs=2, space="PSUM"))

        wp_sb = const.tile([C, HA], bf16, tag="wp")
        nc.sync.dma_start(wp_sb[:], wpd[:])
        w2s_sb = const.tile([HA, 2], bf16, tag="w2s")
        nc.sync.dma_start(w2s_sb[:], w2d[:])
        bf_sb = const.tile([HA, 2], f32, tag="bf")
        nc.sync.dma_start(bf_sb[:], bfd[:])
        b1aug = bf_sb[:, 0:1]   # [b1; 0]
        floor = bf_sb[:, 1:2]   # [0...0; -3e38]

        # Warm-ups: absorb each param DMA's semaphore on its consuming engine
        # so steady-state instructions carry at most one wait.
        warm_ps = psV.tile([P, 1], f32, tag="warm", bufs=1)
        nc.tensor.matmul(
            warm_ps[0:HA, :], wp_sb[:], wp_sb[:, 0:1], start=True, stop=True
        )
        nc.tensor.matmul(
            warm_ps[0:2, :], w2s_sb[:], w2s_sb[:, 0:1], start=True, stop=True
        )
        bf_warm = const.tile([HA, 2], f32, tag="bfw")
        nc.vector.tensor_copy(bf_warm[:], bf_sb[:])

        nsub = T // SUBC
        for i in range(ntiles):
            r0 = i * TILE_ROWS
            x1_t = xin.tile([P, T, C], bf16, tag="x1")
            nc.scalar.dma_start(
                x1_t[:], x1d[r0:r0 + TILE_ROWS, :].rearrange("(p t) c -> p t c", p=P)
            )
            x2_t = xin.tile([P, T, C], bf16, tag="x2")
            nc.sync.dma_start(
                x2_t[:], x2d[r0:r0 + TILE_ROWS, :].rearrange("(p t) c -> p t c", p=P)
            )
            xt1 = xtp.tile([C, TILE_ROWS], bf16, tag="xt1")
            nc.sync.dma_start(xt1[:], x1d[r0:r0 + TILE_ROWS, :], transpose=True)
            xt2 = xtp.tile([C, TILE_ROWS], bf16, tag="xt2")
            nc.scalar.dma_start(xt2[:], x2d[r0:r0 + TILE_ROWS, :], transpose=True)

            relu_sb = [
                relup.tile([HA, T * P], bf16, tag=f"relu{j}", name=f"relu{j}_{i}")
                for j in range(2)
            ]
            s_ps = psS.tile([P, T], f32, tag="s")

            for j, xtj in enumerate((xt1, xt2)):
                for s in range(nsub):
                    cols = slice(s * SUBC * P, (s + 1) * SUBC * P)
                    vp = psV.tile([HA, SUBC * P], f32, tag="v")
                    nc.tensor.matmul(
                        vp[:], wp_sb[:], xtj[:, cols], start=True, stop=True
                    )
                    # rows 0..63: relu(v + b1); row 64: passthrough (floor -inf)
                    nc.vector.tensor_scalar(
                        relu_sb[j][:, cols], vp[:], b1aug, floor, ALU.add, ALU.max
                    )
            # scores accumulate d = s1 - s2 directly (x2 weights host-negated).
            # relu cols are in DMA-transpose (DRAM-row) order r = t*128 + p;
            # the strided lhsT slice [h, t, :] remaps output partition p to
            # row p*T + t, matching the natural-side block layout.
            relu_v = [
                relu_sb[j][:].rearrange("h (p t) -> h t p", t=T) for j in range(2)
            ]
            for t in range(T):
                for j in range(2):
                    nc.tensor.matmul(
                        s_ps[:, t:t + 1],
                        relu_v[j][:, t, :],
                        w2s_sb[:, j:j + 1],
                        start=(j == 0),
                        stop=(j == 1),
                        skip_group_check=True,
                    )

            # beta1 = 0.5*tanh(d) + 0.5 (0.5 score scale baked into weights)
            d_sb = small.tile([P, T], f32, tag="d")
            nc.scalar.activation(d_sb[:], s_ps[:], AF.Tanh)
            bo = small.tile([P, T, 2], f32, tag="bo")
            nc.vector.tensor_scalar(
                bo[:, :, 0], d_sb[:], 0.5, 0.5, ALU.mult, ALU.add
            )
            nc.vector.tensor_scalar(
                bo[:, :, 1], d_sb[:], -0.5, 0.5, ALU.mult, ALU.add
            )

            # z = tanh(beta * x): one broadcast tensor_tensor per input, then
            # one big tanh
            zq = zp.tile([P, T, 2 * C], bf16, tag="zq")
            nc.vector.tensor_tensor(
                zq[:, :, 0:C],
                x1_t[:],
                bo[:, :, 0:1].broadcast_to([P, T, C]),
                ALU.mult,
            )
            nc.vector.tensor_tensor(
                zq[:, :, C:2 * C],
                x2_t[:],
                bo[:, :, 1:2].broadcast_to([P, T, C]),
                ALU.mult,
            )
            z_sb = zp.tile([P, T, 2 * C], f32, tag="z")
            nc.scalar.activation(z_sb[:], zq[:], AF.Tanh)

            nc.scalar.dma_start(
                zd[r0:r0 + TILE_ROWS, :].rearrange("(p t) c -> p t c", p=P),
                z_sb[:],
            )
            nc.sync.dma_start(
                bd[r0:r0 + TILE_ROWS, :].rearrange("(p t) k -> p t k", p=P),
                bo[:],
            )
    nc.compile()
    return nc


def _host_params(W1, b1, W2, b2):
    """Pre-transform the tiny MLP params on host (pure numpy)."""
    W1 = np.asarray(W1, dtype=np.float32)
    W2 = np.asarray(W2, dtype=np.float32).reshape(H, 1)
    b1 = np.asarray(b1, dtype=np.float32).reshape(H, 1)
    # 0.5 folds the tanh-sigmoid half-argument into all score weights
    g = (0.5 * 0.01) * (W1 @ W2)                    # [C, 1]
    w2r = (0.5 * 0.99) * W2                         # [H, 1]
    wpaug = np.concatenate([W1, g], axis=1).astype(ml_dtypes.bfloat16)
    w2a = np.concatenate([w2r, [[1.0]]], axis=0).astype(ml_dtypes.bfloat16)
    w2s = np.concatenate([w2a, -w2a], axis=1)       # [HA, 2]
    b1aug = np.concatenate([b1, [[0.0]]], axis=0).astype(np.float32)
    floor = np.zeros((HA, 1), np.float32)
    floor[H, 0] = -3.0e38
    bfl = np.concatenate([b1aug, floor], axis=1)    # [HA, 2]
    return {
        "wpaug": np.ascontiguousarray(wpaug),
        "w2s": np.ascontiguousarray(w2s),
        "bfl": np.ascontiguousarray(bfl),
    }


def _shard_inputs(x1, x2, params):
    x1 = np.asarray(x1, dtype=np.float32).astype(ml_dtypes.bfloat16)
    x2 = np.asarray(x2, dtype=np.float32).astype(ml_dtypes.bfloat16)
    pad = ROWS_PAD - SHARD
    zpad = np.zeros((pad, C), ml_dtypes.bfloat16)
    in_maps = []
    for c in range(NCORES):
        lo = c * SHARD
        hi = lo + SHARD
        m = {
            "x1b": np.ascontiguousarray(
                np.concatenate([x1[lo:hi], zpad], axis=0)
            ),
            "x2b": np.ascontiguousarray(
                np.concatenate([x2[lo:hi], zpad], axis=0)
            ),
        }
        m.update(params)
        in_maps.append(m)
    return in_maps


def _install_ntff_shim():
    """Register the axon NTFF profile hook if the image's antenv lacks it."""
    import sys
    import types

    try:
        from antenv.axon_hooks import get_axon_ntff_profile_hook  # noqa: F401

        return
    except ImportError:
        pass
    try:
        import antenv
        from trn_agent_boot.trn_boot import _ntff_profile_via_ctypes

        mod = types.ModuleType("antenv.axon_hooks")
        holder = {"h": None}
        mod.set_axon_ntff_profile_hook = lambda h: holder.__setitem__("h", h)
        mod.get_axon_ntff_profile_hook = lambda: holder["h"]
        sys.modules["antenv.axon_hooks"] = mod
        antenv.axon_hooks = mod
        mod.set_axon_ntff_profile_hook(
            _ntff_profile_via_ctypes("/opt/axon/libaxon_pjrt.so")
        )
    except Exception as e:  # profiling is best-effort
        print("ntff shim failed:", e)


def _run(in_maps, trace=False):
    from concourse.bass_utils import run_bass_kernel_spmd

    if trace:
        _install_ntff_shim()
    nc = _build(NTILES)
    res = run_bass_kernel_spmd(
        nc, in_maps, list(range(NCORES)), trace=trace
    )
    return res


def kernel(x1, x2, W1, b1, W2, b2, _trace=False, _return_results=False):
    params = _host_params(W1, b1, W2, b2)
    in_maps = _shard_inputs(x1, x2, params)
    res = _run(in_maps, trace=_trace)
    Z = np.empty((N_TOTAL, 1, 2 * C), dtype=np.float32)
    beta = np.empty((N_TOTAL, 2, 1), dtype=np.float32)
    for c in range(NCORES):
        lo = c * SHARD
        hi = lo + SHARD
        Z[lo:hi, 0, :] = res.results[c]["z"][:SHARD]
        beta[lo:hi, :, 0] = res.results[c]["beta"][:SHARD]
    if _return_results:
        return (Z, beta), res
    return Z, beta


# revision 19
# speedup vs baseline: 1.0839x; 1.0839x over previous
"""Trainium2 Bass kernel for the attention-weighted pair-combine module.

Reference math (per row i of N):
    w1 = leaky_relu(x1 @ W1 + b1, 0.01) @ W2 + b2     [scalar]
    w2 = leaky_relu(x2 @ W1 + b1, 0.01) @ W2 + b2     [scalar]
    (b1, b2) = softmax over {w1, w2}
    Z = tanh([b1*x1, b2*x2])   -> [N, 1, 2C]
    beta = [b1; b2]            -> [N, 2, 1]

Device strategy (pure data-parallel over 8 NeuronCores, see spec sharding_hint):
  - softmax over 2 == sigmoid of the difference: beta1 = 0.5*tanh(0.5*(w1-w2)) + 0.5.
    b2 cancels entirely; only Tanh ACT tables are used -> no table switches.
  - leaky_relu(u)@W2 == 0.01*u@W2 + 0.99*relu(u)@W2. The linear term is x @ g
    with g = 0.01*W1@W2 folded as an extra COLUMN of the v-matmul weights
    (wpaug = [W1 | g]), so it costs nothing. The relu-evac uses a per-partition
    max-floor (0 for relu rows, -3e38 for the linear row) so one fused DVE
    tensor_scalar evacuates relu(v+b1) AND passes the linear row through.
  - scores for x2 use host-negated weights and PSUM-accumulate on top of x1's,
    so the score PSUM holds d = 0.5*(w1-w2) directly.
  - x is pre-cast to bf16 on host; the transposed copy (needed because the PE
    contracts over the partition dim) is loaded by DMA-transpose, which needs
    a 2-byte dtype. No PE transposes, no on-chip casts.
  - rows are tiled 2048 at a time; partition p owns 16 consecutive rows of the
    tile (block mapping) so every DMA descriptor is >= 128B contiguous.
"""

import numpy as np
import ml_dtypes
from contextlib import ExitStack

N_TOTAL = 500000
C = 128
H = 64
HA = H + 1        # v rows: 64 relu features + 1 linear passthrough row
NCORES = 8
P = 128           # partitions
T = 16            # rows per partition per tile (block mapping)
TILE_ROWS = P * T # 2048
SHARD = N_TOTAL // NCORES             # 62500
NTILES = -(-SHARD // TILE_ROWS)       # 31
ROWS_PAD = NTILES * TILE_ROWS         # 63488
SUBC = 4          # chunks (of 128 rows) per PE sub-pass; 4*128 = 512 moving cols


def _build(ntiles):
    import concourse.tile as tile
    from concourse import bacc, mybir

    f32 = mybir.dt.float32
    bf16 = mybir.dt.bfloat16
    AF = mybir.ActivationFunctionType
    ALU = mybir.AluOpType

    rows = ntiles * TILE_ROWS
    # Bacc (not plain Bass): its compile() legalizes multi-sem-wait
    # instructions, which this walrus build rejects outright.
    nc = bacc.Bacc(None, debug=False)
    x1d = nc.dram_tensor("x1b", [rows, C], bf16, kind="ExternalInput")
    x2d = nc.dram_tensor("x2b", [rows, C], bf16, kind="ExternalInput")
    wpd = nc.dram_tensor("wpaug", [C, HA], bf16, kind="ExternalInput")
    w2d = nc.dram_tensor("w2s", [HA, 2], bf16, kind="ExternalInput")
    bfd = nc.dram_tensor("bfl", [HA, 2], f32, kind="ExternalInput")
    zd = nc.dram_tensor("z", [rows, 2 * C], f32, kind="ExternalOutput")
    bd = nc.dram_tensor("beta", [rows, 2], f32, kind="ExternalOutput")

    with ExitStack() as ctx:
        tc = ctx.enter_context(tile.TileContext(nc))
        const = ctx.enter_context(tc.tile_pool(name="const", bufs=1))
        xin = ctx.enter_context(tc.tile_pool(name="xin", bufs=3))
        xtp = ctx.enter_context(tc.tile_pool(name="xt", bufs=3))
        relup = ctx.enter_context(tc.tile_pool(name="relu", bufs=2))
        small = ctx.enter_context(tc.tile_pool(name="small", bufs=2))
        zp = ctx.enter_context(tc.tile_pool(name="zp", bufs=3))
        psV = ctx.enter_context(tc.tile_pool(name="psV", bufs=3, space="PSUM"))
        psS = ctx.enter_context(tc.tile_pool(name="psS", bufs=2, space="PSUM"))

        wp_sb = const.tile([C, HA], bf16, tag="wp")
        nc.sync.dma_start(wp_sb[:], wpd[:])
        w2s_sb = const.tile([HA, 2], bf16, tag="w2s")
        nc.sync.dma_start(w2s_sb[:], w2d[:])
        bf_sb = const.tile([HA, 2], f32, tag="bf")
        nc.sync.dma_start(bf_sb[:], bfd[:])
        b1aug = bf_sb[:, 0:1]   # [b1; 0]
        floor = bf_sb[:, 1:2]   # [0...0; -3e38]

        # Warm-ups: absorb each param DMA's semaphore on its consuming engine
        # so steady-state instructions carry at most one wait.
        warm_ps = psV.tile([P, 1], f32, tag="warm", bufs=1)
        nc.tensor.matmul(
            warm_ps[0:HA, :], wp_sb[:], wp_sb[:, 0:1], start=True, stop=True
        )
        nc.tensor.matmul(
            warm_ps[0:2, :], w2s_sb[:], w2s_sb[:, 0:1], start=True, stop=True
        )
        bf_warm = const.tile([HA, 2], f32, tag="bfw")
        nc.vector.tensor_copy(bf_warm[:], bf_sb[:])

        nsub = T // SUBC
        for i in range(ntiles):
            r0 = i * TILE_ROWS
            x1_t = xin.tile([P, T, C], bf16, tag="x1")
            nc.scalar.dma_start(
                x1_t[:], x1d[r0:r0 + TILE_ROWS, :].rearrange("(p t) c -> p t c", p=P)
            )
            x2_t = xin.tile([P, T, C], bf16, tag="x2")
            nc.scalar.dma_start(
                x2_t[:], x2d[r0:r0 + TILE_ROWS, :].rearrange("(p t) c -> p t c", p=P)
            )
            xt1 = xtp.tile([C, TILE_ROWS], bf16, tag="xt1")
            nc.sync.dma_start(xt1[:], x1d[r0:r0 + TILE_ROWS, :], transpose=True)
            xt2 = xtp.tile([C, TILE_ROWS], bf16, tag="xt2")
            nc.sync.dma_start(xt2[:], x2d[r0:r0 + TILE_ROWS, :], transpose=True)

            relu_sb = [
                relup.tile([HA, T * P], bf16, tag=f"relu{j}", name=f"relu{j}_{i}")
                for j in range(2)
            ]
            s_ps = psS.tile([P, T], f32, tag="s")

            for j, xtj in enumerate((xt1, xt2)):
                for s in range(nsub):
                    cols = slice(s * SUBC * P, (s + 1) * SUBC * P)
                    vp = psV.tile([HA, SUBC * P], f32, tag="v")
                    nc.tensor.matmul(
                        vp[:], wp_sb[:], xtj[:, cols], start=True, stop=True
                    )
                    # rows 0..63: relu(v + b1); row 64: passthrough (floor -inf)
                    nc.vector.tensor_scalar(
                        relu_sb[j][:, cols], vp[:], b1aug, floor, ALU.add, ALU.max
                    )
            # scores accumulate d = s1 - s2 directly (x2 weights host-negated).
            # relu cols are in DMA-transpose (DRAM-row) order r = t*128 + p;
            # the strided lhsT slice [h, t, :] remaps output partition p to
            # row p*T + t, matching the natural-side block layout.
            relu_v = [
                relu_sb[j][:].rearrange("h (p t) -> h t p", t=T) for j in range(2)
            ]
            for t in range(T):
                for j in range(2):
                    nc.tensor.matmul(
                        s_ps[:, t:t + 1],
                        relu_v[j][:, t, :],
                        w2s_sb[:, j:j + 1],
                        start=(j == 0),
                        stop=(j == 1),
                        skip_group_check=True,
                    )

            # beta1 = 0.5*tanh(d) + 0.5 (0.5 score scale baked into weights)
            d_sb = small.tile([P, T], f32, tag="d")
            nc.scalar.activation(d_sb[:], s_ps[:], AF.Tanh)
            bo = small.tile([P, T, 2], f32, tag="bo")
            nc.vector.tensor_scalar(
                bo[:, :, 0], d_sb[:], 0.5, 0.5, ALU.mult, ALU.add
            )
            nc.vector.tensor_scalar(
                bo[:, :, 1], d_sb[:], -0.5, 0.5, ALU.mult, ALU.add
            )

            # z = tanh(beta * x): one broadcast tensor_tensor per input, then
            # one big tanh
            zq = zp.tile([P, T, 2 * C], bf16, tag="zq")
            nc.vector.tensor_tensor(
                zq[:, :, 0:C],
                x1_t[:],
                bo[:, :, 0:1].broadcast_to([P, T, C]),
                ALU.mult,
            )
            nc.vector.tensor_tensor(
                zq[:, :, C:2 * C],
                x2_t[:],
                bo[:, :, 1:2].broadcast_to([P, T, C]),
                ALU.mult,
            )
            z_sb = zp.tile([P, T, 2 * C], f32, tag="z")
            nc.scalar.activation(z_sb[:], zq[:], AF.Tanh)

            nc.scalar.dma_start(
                zd[r0:r0 + TILE_ROWS, :].rearrange("(p t) c -> p t c", p=P),
                z_sb[:],
            )
            nc.sync.dma_start(
                bd[r0:r0 + TILE_ROWS, :].rearrange("(p t) k -> p t k", p=P),
                bo[:],
            )
    nc.compile()
    return nc


def _host_params(W1, b1, W2, b2):
    """Pre-transform the tiny MLP params on host (pure numpy)."""
    W1 = np.asarray(W1, dtype=np.float32)
    W2 = np.asarray(W2, dtype=np.float32).reshape(H, 1)
    b1 = np.asarray(b1, dtype=np.float32).reshape(H, 1)
    # 0.5 folds the tanh-sigmoid half-argument into all score weights
    g = (0.5 * 0.01) * (W1 @ W2)                    # [C, 1]
    w2r = (0.5 * 0.99) * W2                         # [H, 1]
    wpaug = np.concatenate([W1, g], axis=1).astype(ml_dtypes.bfloat16)
    w2a = np.concatenate([w2r, [[1.0]]], axis=0).astype(ml_dtypes.bfloat16)
    w2s = np.concatenate([w2a, -w2a], axis=1)       # [HA, 2]
    b1aug = np.concatenate([b1, [[0.0]]], axis=0).astype(np.float32)
    floor = np.zeros((HA, 1), np.float32)
    floor[H, 0] = -3.0e38
    bfl = np.concatenate([b1aug, floor], axis=1)    # [HA, 2]
    return {
        "wpaug": np.ascontiguousarray(wpaug),
        "w2s": np.ascontiguousarray(w2s),
        "bfl": np.ascontiguousarray(bfl),
    }


def _shard_inputs(x1, x2, params):
    x1 = np.asarray(x1, dtype=np.float32).astype(ml_dtypes.bfloat16)
    x2 = np.asarray(x2, dtype=np.float32).astype(ml_dtypes.bfloat16)
    pad = ROWS_PAD - SHARD
    zpad = np.zeros((pad, C), ml_dtypes.bfloat16)
    in_maps = []
    for c in range(NCORES):
        lo = c * SHARD
        hi = lo + SHARD
        m = {
            "x1b": np.ascontiguousarray(
                np.concatenate([x1[lo:hi], zpad], axis=0)
            ),
            "x2b": np.ascontiguousarray(
                np.concatenate([x2[lo:hi], zpad], axis=0)
            ),
        }
        m.update(params)
        in_maps.append(m)
    return in_maps


def _install_ntff_shim():
    """Register the axon NTFF profile hook if the image's antenv lacks it."""
    import sys
    import types

    try:
        from antenv.axon_hooks import get_axon_ntff_profile_hook  # noqa: F401

        return
    except ImportError:
        pass
    try:
        import antenv
        from trn_agent_boot.trn_boot import _ntff_profile_via_ctypes

        mod = types.ModuleType("antenv.axon_hooks")
        holder = {"h": None}
        mod.set_axon_ntff_profile_hook = lambda h: holder.__setitem__("h", h)
        mod.get_axon_ntff_profile_hook = lambda: holder["h"]
        sys.modules["antenv.axon_hooks"] = mod
        antenv.axon_hooks = mod
        mod.set_axon_ntff_profile_hook(
            _ntff_profile_via_ctypes("/opt/axon/libaxon_pjrt.so")
        )
    except Exception as e:  # profiling is best-effort
        print("ntff shim failed:", e)


def _run(in_maps, trace=False):
    from concourse.bass_utils import run_bass_kernel_spmd

    if trace:
        _install_ntff_shim()
    nc = _build(NTILES)
    res = run_bass_kernel_spmd(
        nc, in_maps, list(range(NCORES)), trace=trace
    )
    return res


def kernel(x1, x2, W1, b1, W2, b2, _trace=False, _return_results=False):
    params = _host_params(W1, b1, W2, b2)
    in_maps = _shard_inputs(x1, x2, params)
    res = _run(in_maps, trace=_trace)
    Z = np.empty((N_TOTAL, 1, 2 * C), dtype=np.float32)
    beta = np.empty((N_TOTAL, 2, 1), dtype=np.float32)
    for c in range(NCORES):
        lo = c * SHARD
        hi = lo + SHARD
        Z[lo:hi, 0, :] = res.results[c]["z"][:SHARD]
        beta[lo:hi, :, 0] = res.results[c]["beta"][:SHARD]
    if _return_results:
        return (Z, beta), res
    return Z, beta


# revision 20
# speedup vs baseline: 1.1642x; 1.0741x over previous
"""Trainium2 Bass kernel for the attention-weighted pair-combine module.

Reference math (per row i of N):
    w1 = leaky_relu(x1 @ W1 + b1, 0.01) @ W2 + b2     [scalar]
    w2 = leaky_relu(x2 @ W1 + b1, 0.01) @ W2 + b2     [scalar]
    (b1, b2) = softmax over {w1, w2}
    Z = tanh([b1*x1, b2*x2])   -> [N, 1, 2C]
    beta = [b1; b2]            -> [N, 2, 1]

Device strategy (pure data-parallel over 8 NeuronCores, see spec sharding_hint):
  - softmax over 2 == sigmoid of the difference: beta1 = 0.5*tanh(0.5*(w1-w2)) + 0.5.
    b2 cancels entirely; only Tanh ACT tables are used -> no table switches.
  - leaky_relu(u)@W2 == 0.01*u@W2 + 0.99*relu(u)@W2. The linear term is x @ g
    with g = 0.01*W1@W2 folded as an extra COLUMN of the v-matmul weights
    (wpaug = [W1 | g]), so it costs nothing. The relu-evac uses a per-partition
    max-floor (0 for relu rows, -3e38 for the linear row) so one fused DVE
    tensor_scalar evacuates relu(v+b1) AND passes the linear row through.
  - scores for x2 use host-negated weights and PSUM-accumulate on top of x1's,
    so the score PSUM holds d = 0.5*(w1-w2) directly.
  - x is pre-cast to bf16 on host; the transposed copy (needed because the PE
    contracts over the partition dim) is loaded by DMA-transpose, which needs
    a 2-byte dtype. No PE transposes, no on-chip casts.
  - rows are tiled 2048 at a time; partition p owns 16 consecutive rows of the
    tile (block mapping) so every DMA descriptor is >= 128B contiguous.
"""

import numpy as np
import ml_dtypes
from contextlib import ExitStack

N_TOTAL = 500000
C = 128
H = 64
HA = H + 1        # v rows: 64 relu features + 1 linear passthrough row
NCORES = 8
P = 128           # partitions
T = 16            # rows per partition per tile (block mapping)
TILE_ROWS = P * T # 2048
SHARD = N_TOTAL // NCORES             # 62500
NTILES = -(-SHARD // TILE_ROWS)       # 31
ROWS_PAD = NTILES * TILE_ROWS         # 63488
SUBC = 4          # chunks (of 128 rows) per PE sub-pass; 4*128 = 512 moving cols


def _build(ntiles):
    import concourse.tile as tile
    from concourse import bacc, mybir

    f32 = mybir.dt.float32
    bf16 = mybir.dt.bfloat16
    AF = mybir.ActivationFunctionType
    ALU = mybir.AluOpType

    rows = ntiles * TILE_ROWS
    # Bacc (not plain Bass): its compile() legalizes multi-sem-wait
    # instructions, which this walrus build rejects outright.
    nc = bacc.Bacc(None, debug=False)
    x1d = nc.dram_tensor("x1b", [rows, C], bf16, kind="ExternalInput")
    x2d = nc.dram_tensor("x2b", [rows, C], bf16, kind="ExternalInput")
    wpd = nc.dram_tensor("wpaug", [C, HA], bf16, kind="ExternalInput")
    w2d = nc.dram_tensor("w2s", [HA, 2], bf16, kind="ExternalInput")
    bfd = nc.dram_tensor("bfl", [HA, 2], f32, kind="ExternalInput")
    zd = nc.dram_tensor("z", [rows, 2 * C], bf16, kind="ExternalOutput")
    bd = nc.dram_tensor("beta", [rows, 2], bf16, kind="ExternalOutput")

    with ExitStack() as ctx:
        tc = ctx.enter_context(tile.TileContext(nc))
        const = ctx.enter_context(tc.tile_pool(name="const", bufs=1))
        xin = ctx.enter_context(tc.tile_pool(name="xin", bufs=2))
        xtp = ctx.enter_context(tc.tile_pool(name="xt", bufs=2))
        relup = ctx.enter_context(tc.tile_pool(name="relu", bufs=2))
        small = ctx.enter_context(tc.tile_pool(name="small", bufs=2))
        zp = ctx.enter_context(tc.tile_pool(name="zp", bufs=2))
        psV = ctx.enter_context(tc.tile_pool(name="psV", bufs=2, space="PSUM"))
        psS = ctx.enter_context(tc.tile_pool(name="psS", bufs=2, space="PSUM"))

        wp_sb = const.tile([C, HA], bf16, tag="wp")
        nc.sync.dma_start(wp_sb[:], wpd[:])
        w2s_sb = const.tile([HA, 2], bf16, tag="w2s")
        nc.sync.dma_start(w2s_sb[:], w2d[:])
        bf_sb = const.tile([HA, 2], f32, tag="bf")
        nc.sync.dma_start(bf_sb[:], bfd[:])
        b1aug = bf_sb[:, 0:1]   # [b1; 0]
        floor = bf_sb[:, 1:2]   # [0...0; -3e38]

        # Warm-ups: absorb each param DMA's semaphore on its consuming engine
        # so steady-state instructions carry at most one wait.
        warm_ps = psV.tile([P, 1], f32, tag="warm", bufs=1)
        nc.tensor.matmul(
            warm_ps[0:HA, :], wp_sb[:], wp_sb[:, 0:1], start=True, stop=True
        )
        nc.tensor.matmul(
            warm_ps[0:2, :], w2s_sb[:], w2s_sb[:, 0:1], start=True, stop=True
        )
        bf_warm = const.tile([HA, 2], f32, tag="bfw")
        nc.vector.tensor_copy(bf_warm[:], bf_sb[:])

        nsub = T // SUBC
        for i in range(ntiles):
            r0 = i * TILE_ROWS
            x1_t = xin.tile([P, T, C], bf16, tag="x1")
            nc.scalar.dma_start(
                x1_t[:], x1d[r0:r0 + TILE_ROWS, :].rearrange("(p t) c -> p t c", p=P)
            )
            x2_t = xin.tile([P, T, C], bf16, tag="x2")
            nc.scalar.dma_start(
                x2_t[:], x2d[r0:r0 + TILE_ROWS, :].rearrange("(p t) c -> p t c", p=P)
            )
            xt1 = xtp.tile([C, TILE_ROWS], bf16, tag="xt1")
            nc.sync.dma_start(xt1[:], x1d[r0:r0 + TILE_ROWS, :], transpose=True)
            xt2 = xtp.tile([C, TILE_ROWS], bf16, tag="xt2")
            nc.sync.dma_start(xt2[:], x2d[r0:r0 + TILE_ROWS, :], transpose=True)

            relu_sb = [
                relup.tile([HA, T * P], bf16, tag=f"relu{j}", name=f"relu{j}_{i}")
                for j in range(2)
            ]
            s_ps = psS.tile([P, T], f32, tag="s")

            for j, xtj in enumerate((xt1, xt2)):
                for s in range(nsub):
                    cols = slice(s * SUBC * P, (s + 1) * SUBC * P)
                    vp = psV.tile([HA, SUBC * P], f32, tag="v")
                    nc.tensor.matmul(
                        vp[:], wp_sb[:], xtj[:, cols], start=True, stop=True
                    )
                    # rows 0..63: relu(v + b1); row 64: passthrough (floor -inf)
                    nc.vector.tensor_scalar(
                        relu_sb[j][:, cols], vp[:], b1aug, floor, ALU.add, ALU.max
                    )
            # scores accumulate d = s1 - s2 directly (x2 weights host-negated).
            # relu cols are in DMA-transpose (DRAM-row) order r = t*128 + p;
            # the strided lhsT slice [h, t, :] remaps output partition p to
            # row p*T + t, matching the natural-side block layout.
            relu_v = [
                relu_sb[j][:].rearrange("h (p t) -> h t p", t=T) for j in range(2)
            ]
            for t in range(T):
                for j in range(2):
                    nc.tensor.matmul(
                        s_ps[:, t:t + 1],
                        relu_v[j][:, t, :],
                        w2s_sb[:, j:j + 1],
                        start=(j == 0),
                        stop=(j == 1),
                        skip_group_check=True,
                    )

            # beta1 = 0.5*tanh(d) + 0.5 (0.5 score scale baked into weights)
            d_sb = small.tile([P, T], f32, tag="d")
            nc.scalar.activation(d_sb[:], s_ps[:], AF.Tanh)
            bo = small.tile([P, T, 2], bf16, tag="bo")
            nc.vector.tensor_scalar(
                bo[:, :, 0], d_sb[:], 0.5, 0.5, ALU.mult, ALU.add
            )
            nc.vector.tensor_scalar(
                bo[:, :, 1], d_sb[:], -0.5, 0.5, ALU.mult, ALU.add
            )

            # z = tanh(beta * x): one broadcast tensor_tensor per input, then
            # one big tanh
            zq = zp.tile([P, T, 2 * C], bf16, tag="zq")
            nc.vector.tensor_tensor(
                zq[:, :, 0:C],
                x1_t[:],
                bo[:, :, 0:1].broadcast_to([P, T, C]),
                ALU.mult,
            )
            nc.vector.tensor_tensor(
                zq[:, :, C:2 * C],
                x2_t[:],
                bo[:, :, 1:2].broadcast_to([P, T, C]),
                ALU.mult,
            )
            z_sb = zp.tile([P, T, 2 * C], bf16, tag="z")
            nc.scalar.activation(z_sb[:], zq[:], AF.Tanh)

            nc.scalar.dma_start(
                zd[r0:r0 + TILE_ROWS, :].rearrange("(p t) c -> p t c", p=P),
                z_sb[:],
            )
            nc.sync.dma_start(
                bd[r0:r0 + TILE_ROWS, :].rearrange("(p t) k -> p t k", p=P),
                bo[:],
            )
    nc.compile()
    return nc


def _host_params(W1, b1, W2, b2):
    """Pre-transform the tiny MLP params on host (pure numpy)."""
    W1 = np.asarray(W1, dtype=np.float32)
    W2 = np.asarray(W2, dtype=np.float32).reshape(H, 1)
    b1 = np.asarray(b1, dtype=np.float32).reshape(H, 1)
    # 0.5 folds the tanh-sigmoid half-argument into all score weights
    g = (0.5 * 0.01) * (W1 @ W2)                    # [C, 1]
    w2r = (0.5 * 0.99) * W2                         # [H, 1]
    wpaug = np.concatenate([W1, g], axis=1).astype(ml_dtypes.bfloat16)
    w2a = np.concatenate([w2r, [[1.0]]], axis=0).astype(ml_dtypes.bfloat16)
    w2s = np.concatenate([w2a, -w2a], axis=1)       # [HA, 2]
    b1aug = np.concatenate([b1, [[0.0]]], axis=0).astype(np.float32)
    floor = np.zeros((HA, 1), np.float32)
    floor[H, 0] = -3.0e38
    bfl = np.concatenate([b1aug, floor], axis=1)    # [HA, 2]
    return {
        "wpaug": np.ascontiguousarray(wpaug),
        "w2s": np.ascontiguousarray(w2s),
        "bfl": np.ascontiguousarray(bfl),
    }


def _shard_inputs(x1, x2, params):
    x1 = np.asarray(x1, dtype=np.float32).astype(ml_dtypes.bfloat16)
    x2 = np.asarray(x2, dtype=np.float32).astype(ml_dtypes.bfloat16)
    pad = ROWS_PAD - SHARD
    zpad = np.zeros((pad, C), ml_dtypes.bfloat16)
    in_maps = []
    for c in range(NCORES):
        lo = c * SHARD
        hi = lo + SHARD
        m = {
            "x1b": np.ascontiguousarray(
                np.concatenate([x1[lo:hi], zpad], axis=0)
            ),
            "x2b": np.ascontiguousarray(
                np.concatenate([x2[lo:hi], zpad], axis=0)
            ),
        }
        m.update(params)
        in_maps.append(m)
    return in_maps


def _install_ntff_shim():
    """Register the axon NTFF profile hook if the image's antenv lacks it."""
    import sys
    import types

    try:
        from antenv.axon_hooks import get_axon_ntff_profile_hook  # noqa: F401

        return
    except ImportError:
        pass
    try:
        import antenv
        from trn_agent_boot.trn_boot import _ntff_profile_via_ctypes

        mod = types.ModuleType("antenv.axon_hooks")
        holder = {"h": None}
        mod.set_axon_ntff_profile_hook = lambda h: holder.__setitem__("h", h)
        mod.get_axon_ntff_profile_hook = lambda: holder["h"]
        sys.modules["antenv.axon_hooks"] = mod
        antenv.axon_hooks = mod
        mod.set_axon_ntff_profile_hook(
            _ntff_profile_via_ctypes("/opt/axon/libaxon_pjrt.so")
        )
    except Exception as e:  # profiling is best-effort
        print("ntff shim failed:", e)


def _run(in_maps, trace=False):
    from concourse.bass_utils import run_bass_kernel_spmd

    if trace:
        _install_ntff_shim()
    nc = _build(NTILES)
    res = run_bass_kernel_spmd(
        nc, in_maps, list(range(NCORES)), trace=trace
    )
    return res


def kernel(x1, x2, W1, b1, W2, b2, _trace=False, _return_results=False):
    params = _host_params(W1, b1, W2, b2)
    in_maps = _shard_inputs(x1, x2, params)
    res = _run(in_maps, trace=_trace)
    Z = np.empty((N_TOTAL, 1, 2 * C), dtype=np.float32)
    beta = np.empty((N_TOTAL, 2, 1), dtype=np.float32)
    for c in range(NCORES):
        lo = c * SHARD
        hi = lo + SHARD
        Z[lo:hi, 0, :] = res.results[c]["z"][:SHARD].astype(np.float32)
        beta[lo:hi, :, 0] = res.results[c]["beta"][:SHARD].astype(np.float32)
    if _return_results:
        return (Z, beta), res
    return Z, beta


# revision 21
# speedup vs baseline: 1.2907x; 1.1086x over previous
"""Trainium2 Bass kernel for the attention-weighted pair-combine module.

Reference math (per row i of N):
    w1 = leaky_relu(x1 @ W1 + b1, 0.01) @ W2 + b2     [scalar]
    w2 = leaky_relu(x2 @ W1 + b1, 0.01) @ W2 + b2     [scalar]
    (b1, b2) = softmax over {w1, w2}
    Z = tanh([b1*x1, b2*x2])   -> [N, 1, 2C]
    beta = [b1; b2]            -> [N, 2, 1]

Device strategy (pure data-parallel over 8 NeuronCores, see spec sharding_hint):
  - softmax over 2 == sigmoid of the difference: beta1 = 0.5*tanh(0.5*(w1-w2)) + 0.5.
    b2 cancels entirely; only Tanh ACT tables are used -> no table switches.
  - leaky_relu(u)@W2 == 0.01*u@W2 + 0.99*relu(u)@W2. The linear term is x @ g
    with g = 0.01*W1@W2 folded as an extra COLUMN of the v-matmul weights
    (wpaug = [W1 | g]), so it costs nothing. The relu-evac uses a per-partition
    max-floor (0 for relu rows, -3e38 for the linear row) so one fused DVE
    tensor_scalar evacuates relu(v+b1) AND passes the linear row through.
  - scores for x2 use host-negated weights and PSUM-accumulate on top of x1's,
    so the score PSUM holds d = 0.5*(w1-w2) directly.
  - x is pre-cast to bf16 on host; the transposed copy (needed because the PE
    contracts over the partition dim) is loaded by DMA-transpose, which needs
    a 2-byte dtype. No PE transposes, no on-chip casts.
  - rows are tiled 2048 at a time; partition p owns 16 consecutive rows of the
    tile (block mapping) so every DMA descriptor is >= 128B contiguous.
"""

import numpy as np
import ml_dtypes
from contextlib import ExitStack

N_TOTAL = 500000
C = 128
H = 64
HA = H + 1        # v rows: 64 relu features + 1 linear passthrough row
NCORES = 8
P = 128           # partitions
T = 16            # rows per partition per tile (block mapping)
TILE_ROWS = P * T # 2048
SHARD = N_TOTAL // NCORES             # 62500
NTILES = -(-SHARD // TILE_ROWS)       # 31
ROWS_PAD = NTILES * TILE_ROWS         # 63488
SUBC = 4          # chunks (of 128 rows) per PE sub-pass; 4*128 = 512 moving cols


def _build(ntiles):
    import concourse.tile as tile
    from concourse import bacc, mybir

    f32 = mybir.dt.float32
    bf16 = mybir.dt.bfloat16
    AF = mybir.ActivationFunctionType
    ALU = mybir.AluOpType

    rows = ntiles * TILE_ROWS
    # Bacc (not plain Bass): its compile() legalizes multi-sem-wait
    # instructions, which this walrus build rejects outright.
    nc = bacc.Bacc(None, debug=False)
    x1d = nc.dram_tensor("x1b", [rows, C], bf16, kind="ExternalInput")
    x2d = nc.dram_tensor("x2b", [rows, C], bf16, kind="ExternalInput")
    wpd = nc.dram_tensor("wpaug", [C, HA], bf16, kind="ExternalInput")
    w2d = nc.dram_tensor("w2s", [HA, 2], bf16, kind="ExternalInput")
    bfd = nc.dram_tensor("bfl", [HA, 2], f32, kind="ExternalInput")
    zd = nc.dram_tensor("z", [rows, 2 * C], bf16, kind="ExternalOutput")
    bd = nc.dram_tensor("beta", [rows, 2], bf16, kind="ExternalOutput")

    with ExitStack() as ctx:
        tc = ctx.enter_context(tile.TileContext(nc))
        const = ctx.enter_context(tc.tile_pool(name="const", bufs=1))
        xin = ctx.enter_context(tc.tile_pool(name="xin", bufs=4))
        xtp = ctx.enter_context(tc.tile_pool(name="xt", bufs=3))
        relup = ctx.enter_context(tc.tile_pool(name="relu", bufs=2))
        small = ctx.enter_context(tc.tile_pool(name="small", bufs=2))
        zp = ctx.enter_context(tc.tile_pool(name="zp", bufs=3))
        psV = ctx.enter_context(tc.tile_pool(name="psV", bufs=2, space="PSUM"))
        psS = ctx.enter_context(tc.tile_pool(name="psS", bufs=2, space="PSUM"))

        wp_sb = const.tile([C, HA], bf16, tag="wp")
        nc.sync.dma_start(wp_sb[:], wpd[:])
        w2s_sb = const.tile([HA, 2], bf16, tag="w2s")
        nc.sync.dma_start(w2s_sb[:], w2d[:])
        bf_sb = const.tile([HA, 2], f32, tag="bf")
        nc.sync.dma_start(bf_sb[:], bfd[:])
        b1aug = bf_sb[:, 0:1]   # [b1; 0]
        floor = bf_sb[:, 1:2]   # [0...0; -3e38]

        # Warm-ups: absorb each param DMA's semaphore on its consuming engine
        # so steady-state instructions carry at most one wait.
        warm_ps = psV.tile([P, 1], f32, tag="warm", bufs=1)
        nc.tensor.matmul(
            warm_ps[0:HA, :], wp_sb[:], wp_sb[:, 0:1], start=True, stop=True
        )
        nc.tensor.matmul(
            warm_ps[0:2, :], w2s_sb[:], w2s_sb[:, 0:1], start=True, stop=True
        )
        bf_warm = const.tile([HA, 2], f32, tag="bfw")
        nc.vector.tensor_copy(bf_warm[:], bf_sb[:])

        nsub = T // SUBC
        for i in range(ntiles):
            r0 = i * TILE_ROWS
            x1_t = xin.tile([P, T, C], bf16, tag="x1")
            nc.scalar.dma_start(
                x1_t[:], x1d[r0:r0 + TILE_ROWS, :].rearrange("(p t) c -> p t c", p=P)
            )
            x2_t = xin.tile([P, T, C], bf16, tag="x2")
            nc.scalar.dma_start(
                x2_t[:], x2d[r0:r0 + TILE_ROWS, :].rearrange("(p t) c -> p t c", p=P)
            )
            xt1 = xtp.tile([C, TILE_ROWS], bf16, tag="xt1")
            nc.sync.dma_start(xt1[:], x1d[r0:r0 + TILE_ROWS, :], transpose=True)
            xt2 = xtp.tile([C, TILE_ROWS], bf16, tag="xt2")
            nc.sync.dma_start(xt2[:], x2d[r0:r0 + TILE_ROWS, :], transpose=True)

            relu_sb = [
                relup.tile([HA, T * P], bf16, tag=f"relu{j}", name=f"relu{j}_{i}")
                for j in range(2)
            ]
            s_ps = psS.tile([P, T], f32, tag="s")

            for j, xtj in enumerate((xt1, xt2)):
                for s in range(nsub):
                    cols = slice(s * SUBC * P, (s + 1) * SUBC * P)
                    vp = psV.tile([HA, SUBC * P], f32, tag="v")
                    nc.tensor.matmul(
                        vp[:], wp_sb[:], xtj[:, cols], start=True, stop=True
                    )
                    # rows 0..63: relu(v + b1); row 64: passthrough (floor -inf)
                    nc.vector.tensor_scalar(
                        relu_sb[j][:, cols], vp[:], b1aug, floor, ALU.add, ALU.max
                    )
            # scores accumulate d = s1 - s2 directly (x2 weights host-negated).
            # relu cols are in DMA-transpose (DRAM-row) order r = t*128 + p;
            # the strided lhsT slice [h, t, :] remaps output partition p to
            # row p*T + t, matching the natural-side block layout.
            relu_v = [
                relu_sb[j][:].rearrange("h (p t) -> h t p", t=T) for j in range(2)
            ]
            for t in range(T):
                for j in range(2):
                    nc.tensor.matmul(
                        s_ps[:, t:t + 1],
                        relu_v[j][:, t, :],
                        w2s_sb[:, j:j + 1],
                        start=(j == 0),
                        stop=(j == 1),
                        skip_group_check=True,
                    )

            # beta1 = 0.5*tanh(d) + 0.5 (0.5 score scale baked into weights)
            d_sb = small.tile([P, T], f32, tag="d")
            nc.scalar.activation(d_sb[:], s_ps[:], AF.Tanh)
            bo = small.tile([P, T, 2], bf16, tag="bo")
            nc.vector.tensor_scalar(
                bo[:, :, 0], d_sb[:], 0.5, 0.5, ALU.mult, ALU.add
            )
            nc.vector.tensor_scalar(
                bo[:, :, 1], d_sb[:], -0.5, 0.5, ALU.mult, ALU.add
            )

            # z = tanh(beta * x): one broadcast tensor_tensor per input, then
            # one big tanh
            zq = zp.tile([P, T, 2 * C], bf16, tag="zq")
            nc.vector.tensor_tensor(
                zq[:, :, 0:C],
                x1_t[:],
                bo[:, :, 0:1].broadcast_to([P, T, C]),
                ALU.mult,
            )
            nc.vector.tensor_tensor(
                zq[:, :, C:2 * C],
                x2_t[:],
                bo[:, :, 1:2].broadcast_to([P, T, C]),
                ALU.mult,
            )
            z_sb = zp.tile([P, T, 2 * C], bf16, tag="z")
            nc.scalar.activation(z_sb[:], zq[:], AF.Tanh)

            nc.gpsimd.dma_start(
                zd[r0:r0 + TILE_ROWS, :].rearrange("(p t) c -> p t c", p=P),
                z_sb[:],
            )
            nc.gpsimd.dma_start(
                bd[r0:r0 + TILE_ROWS, :].rearrange("(p t) k -> p t k", p=P),
                bo[:],
            )
    nc.compile()
    return nc


def _host_params(W1, b1, W2, b2):
    """Pre-transform the tiny MLP params on host (pure numpy)."""
    W1 = np.asarray(W1, dtype=np.float32)
    W2 = np.asarray(W2, dtype=np.float32).reshape(H, 1)
    b1 = np.asarray(b1, dtype=np.float32).reshape(H, 1)
    # 0.5 folds the tanh-sigmoid half-argument into all score weights
    g = (0.5 * 0.01) * (W1 @ W2)                    # [C, 1]
    w2r = (0.5 * 0.99) * W2                         # [H, 1]
    wpaug = np.concatenate([W1, g], axis=1).astype(ml_dtypes.bfloat16)
    w2a = np.concatenate([w2r, [[1.0]]], axis=0).astype(ml_dtypes.bfloat16)
    w2s = np.concatenate([w2a, -w2a], axis=1)       # [HA, 2]
    b1aug = np.concatenate([b1, [[0.0]]], axis=0).astype(np.float32)
    floor = np.zeros((HA, 1), np.float32)
    floor[H, 0] = -3.0e38
    bfl = np.concatenate([b1aug, floor], axis=1)    # [HA, 2]
    return {
        "wpaug": np.ascontiguousarray(wpaug),
        "w2s": np.ascontiguousarray(w2s),
        "bfl": np.ascontiguousarray(bfl),
    }


def _shard_inputs(x1, x2, params):
    x1 = np.asarray(x1, dtype=np.float32).astype(ml_dtypes.bfloat16)
    x2 = np.asarray(x2, dtype=np.float32).astype(ml_dtypes.bfloat16)
    pad = ROWS_PAD - SHARD
    zpad = np.zeros((pad, C), ml_dtypes.bfloat16)
    in_maps = []
    for c in range(NCORES):
        lo = c * SHARD
        hi = lo + SHARD
        m = {
            "x1b": np.ascontiguousarray(
                np.concatenate([x1[lo:hi], zpad], axis=0)
            ),
            "x2b": np.ascontiguousarray(
                np.concatenate([x2[lo:hi], zpad], axis=0)
            ),
        }
        m.update(params)
        in_maps.append(m)
    return in_maps


def _install_ntff_shim():
    """Register the axon NTFF profile hook if the image's antenv lacks it."""
    import sys
    import types

    try:
        from antenv.axon_hooks import get_axon_ntff_profile_hook  # noqa: F401

        return
    except ImportError:
        pass
    try:
        import antenv
        from trn_agent_boot.trn_boot import _ntff_profile_via_ctypes

        mod = types.ModuleType("antenv.axon_hooks")
        holder = {"h": None}
        mod.set_axon_ntff_profile_hook = lambda h: holder.__setitem__("h", h)
        mod.get_axon_ntff_profile_hook = lambda: holder["h"]
        sys.modules["antenv.axon_hooks"] = mod
        antenv.axon_hooks = mod
        mod.set_axon_ntff_profile_hook(
            _ntff_profile_via_ctypes("/opt/axon/libaxon_pjrt.so")
        )
    except Exception as e:  # profiling is best-effort
        print("ntff shim failed:", e)


def _run(in_maps, trace=False):
    from concourse.bass_utils import run_bass_kernel_spmd

    if trace:
        _install_ntff_shim()
    nc = _build(NTILES)
    res = run_bass_kernel_spmd(
        nc, in_maps, list(range(NCORES)), trace=trace
    )
    return res


def kernel(x1, x2, W1, b1, W2, b2, _trace=False, _return_results=False):
    params = _host_params(W1, b1, W2, b2)
    in_maps = _shard_inputs(x1, x2, params)
    res = _run(in_maps, trace=_trace)
    Z = np.empty((N_TOTAL, 1, 2 * C), dtype=np.float32)
    beta = np.empty((N_TOTAL, 2, 1), dtype=np.float32)
    for c in range(NCORES):
        lo = c * SHARD
        hi = lo + SHARD
        Z[lo:hi, 0, :] = res.results[c]["z"][:SHARD].astype(np.float32)
        beta[lo:hi, :, 0] = res.results[c]["beta"][:SHARD].astype(np.float32)
    if _return_results:
        return (Z, beta), res
    return Z, beta
